# revision 55
# baseline (speedup 1.0000x reference)
"""Trainium2 Bass kernel for AllGNN message passing.

Computes, for full inputs:
    h   = x @ W_in + b_in
    deg = adj.sum(axis=1, keepdims=True)
    agg = (adj @ h) / (deg + 1)
    out = agg @ W_cls + b_cls

Key algebra: row scaling commutes with the right matmul, so
    out = (adj @ G)[:, :C] / (deg+1) + b_cls
with G = [x @ W2 + b2 | ones], W2 = W_in @ W_cls, b2 = b_in @ W_cls.
The ones column's product recovers deg.

Sharding: row-shard adj over 8 cores. The adj row-block is shipped
pre-transposed (adjT = adj_blk.T, [N, rows]) and pre-cast to fp8e4 on the
host -- adj is 0/1 so fp8 is exact and HBM traffic drops 4x vs fp32, and
no on-device transpose is needed at all. x is shipped pre-transposed in
bf16 (replicated) so each core computes the full G locally; no collectives.

Main loop: for each 128-row strip of adjT (fp8, streamed once from HBM),
accumulate out.T[c, i] += G[j, c] * adjT[j, i] with the 41-col G tile as
the stationary operand and the fp8 strip as the moving operand (mixed
bf16 x fp8 matmul, fp32 PSUM accumulation). Three persistent PSUM banks
hold out.T chunks [41, 512/512/480]. G production (x @ W2) is interleaved
one chunk ahead of consumption so the PE never waits on phase A.
"""

import numpy as np

import concourse.bass as bass
from concourse import bacc
import concourse.mybir as mybir
import concourse.tile as tile
from concourse.bass_utils import run_bass_kernel_spmd

import ml_dtypes

N_CORES = 8
N_NODES = 12000
IN_CH = 256
HID = 64
N_CLS = 40

ROWS = N_NODES // N_CORES        # 1500 output rows per core
ROWS_PAD = 1504                  # padded i-dim (8B-aligned fp8 lines)
JW = 128                         # j (contraction) tile width
N_JT = -(-N_NODES // JW)         # 94 real j-tiles
N_KT = IN_CH // 128              # 2 k-tiles for x @ W2
GC = N_CLS + 1                   # G columns: [ones | g]
GJT = 8                          # j-tiles per G-production chunk
N_GCH = 12                       # G chunks
JPAD = N_GCH * GJT * JW          # 12288 padded j-dim
GRP = 4                          # j-tiles per adjT strip-group DMA
N_GRP = JPAD // (GRP * JW)       # 24 strip-group DMAs
# out.T chunk layout across the padded i-dim: 3 PSUM banks
PSU_CHUNKS = [(0, 512), (512, 512), (1024, ROWS_PAD - 1024)]


def build_gnn(
    n_cores=N_CORES,
    strip_bufs=16,
    x_bufs=3,
    g_lookahead=1,
):
    f32 = mybir.dt.float32
    bf16 = mybir.dt.bfloat16
    f8 = mybir.dt.float8e4
    mult = mybir.AluOpType.mult
    add = mybir.AluOpType.add

    nc = bacc.Bacc(num_devices=n_cores)

    # adjT pre-grouped on host: [group, partition, tile-in-group, i] so each
    # partition's GRP j-tiles are contiguous (6016 B lines, 128 descs/DMA)
    adjT_h = nc.dram_tensor(
        "adjT", [N_GRP, 128, GRP, ROWS_PAD], f8, kind="ExternalInput"
    )
    xt_h = nc.dram_tensor("x_Ti", [128, N_KT, JPAD], bf16, kind="ExternalInput")
    # all small weights host-packed into one tensor -> one DMA:
    # [128, eye(128) | W_in as (p, t*64+h) | W_cls (64p) | b_in (64p) | b_cls bcast
    #  | stacked eye41 (rows 0-40 and 64-104) for the half-combining transpose]
    WP_W = 128 + 128 + N_CLS + 1 + N_CLS + GC  # 378
    wpack_h = nc.dram_tensor("wpack", [128, WP_W], f32, kind="ExternalInput")
    out_h = nc.dram_tensor("out_blk", [ROWS, N_CLS], f32, kind="ExternalOutput")

    with tile.TileContext(nc) as tc:
        with (
            tc.tile_pool(name="singles", bufs=1) as singles,
            tc.tile_pool(name="gpool", bufs=N_GCH) as g_pool,
            tc.tile_pool(name="xpool", bufs=x_bufs) as x_pool,
            tc.tile_pool(name="spool", bufs=strip_bufs) as strip_pool,
            tc.tile_pool(name="opool", bufs=4) as out_pool,
            tc.tile_pool(name="psum", bufs=1, space="PSUM") as psum_pool,
        ):
            # PE warmup: ~4us of junk matmuls (no DMA deps) so the HAM
            # clock-gate reaches K=8/8 before real work arrives
            wu_a = singles.tile([128, 128], bf16, tag="wu_a")
            nc.vector.memset(wu_a, 0.0)
            wu_b = singles.tile([128, 512], bf16, tag="wu_b")
            nc.vector.memset(wu_b, 0.0)
            for _ in range(16):
                ps_wu = psum_pool.tile([128, 512], f32, tag="g", bufs=3)
                nc.tensor.matmul(ps_wu, lhsT=wu_a, rhs=wu_b, start=True, stop=True)

            # one packed weight DMA, first on the sync ring (FIFO -> lands
            # before the strip-group DMAs hog the SDMA engines)
            wpack = singles.tile([128, WP_W], f32, tag="wpack")
            nc.sync.dma_start(out=wpack, in_=wpack_h[:])
            id_f = wpack[:, 0:128]
            wcls_sb = wpack[:HID, 256 : 256 + N_CLS]
            bin_sb = wpack[:HID, 296:297]
            bcls_sb = wpack[:, 297 : 297 + N_CLS]

            def win_sb(t):  # W_in k-tile [128, 64]
                return wpack[:, 128 + HID * t : 128 + HID * (t + 1)]

            id_b = singles.tile([128, 128], bf16, tag="id_b")
            nc.vector.tensor_copy(id_b, id_f)
            # stacked identity (bf16): transposes U while summing the two
            # col-tile halves (rows 0-40 and 64-104 both carry eye(41))
            id2_b = singles.tile([128, GC], bf16, tag="id2_b")
            nc.vector.tensor_copy(id2_b, wpack[:, 337 : 337 + GC])
            # persistent finalize U tiles, zeroed once so the unused
            # partition rows contribute exact zeros to the transpose
            U_bfs = []
            for ch in range(len(PSU_CHUNKS)):
                U_bf = singles.tile([128, 512], bf16, tag=f"Ubf{ch}", name="Ubf")
                nc.vector.memset(U_bf, 0.0)
                U_bfs.append(U_bf)

            # persistent PSUM banks for the out.T accumulation
            psU = [
                psum_pool.tile([128, 512], f32, tag=f"U{i}", name=f"U{i}", bufs=1)
                for i in range(len(PSU_CHUNKS))
            ]

            # ---- Phase A: W2 = W_in @ W_cls, b2 = b_in @ W_cls (tiny) ----
            ones_sb = singles.tile([1, 128], f32, tag="ones")
            nc.vector.memset(ones_sb, 1.0)

            # W_in.T tiles via PE transpose (fp32)
            winT_sb = singles.tile([HID, N_KT, 128], f32, tag="winT")
            for t in range(N_KT):
                ps_w = psum_pool.tile([128, 512], f32, tag="g", bufs=3)
                ps = ps_w[:HID, :128]
                nc.tensor.matmul(ps, lhsT=win_sb(t), rhs=id_f, start=True, stop=True)
                nc.vector.tensor_copy(winT_sb[:, t, :], ps)
            # W2 = W_in @ W_cls -> bf16
            w2b_sb = singles.tile([128, N_KT, N_CLS], bf16, tag="w2b")
            for t in range(N_KT):
                ps_w = psum_pool.tile([128, 512], f32, tag="g", bufs=3)
                ps = ps_w[:, :N_CLS]
                nc.tensor.matmul(
                    ps, lhsT=winT_sb[:, t, :], rhs=wcls_sb, start=True, stop=True
                )
                nc.vector.tensor_copy(w2b_sb[:, t, :], ps)
            # b2 = b_in @ W_cls broadcast to [128, N_CLS]
            ps_b2w = psum_pool.tile([128, 512], f32, tag="g", bufs=3)
            ps_b2 = ps_b2w[:1, :N_CLS]
            nc.tensor.matmul(ps_b2, lhsT=bin_sb, rhs=wcls_sb, start=True, stop=True)
            b2row = singles.tile([1, N_CLS], f32, tag="b2row")
            nc.vector.tensor_copy(b2row, ps_b2)
            ps_b2bw = psum_pool.tile([128, 512], f32, tag="g", bufs=3)
            ps_b2b = ps_b2bw[:, :N_CLS]
            nc.tensor.matmul(ps_b2b, lhsT=ones_sb, rhs=b2row, start=True, stop=True)
            b2b_sb = singles.tile([128, N_CLS], f32, tag="b2b")
            nc.vector.tensor_copy(b2b_sb, ps_b2b)

            # ---- G production: one chunk = GJT j-tiles of G = x @ W2 + b2.
            # G columns: [ones | g0..g39] -- ones first so deg lands on
            # PSUM partition 0 in the out.T accumulation.
            G_tiles = {}

            def g_job(q):
                if q >= N_GCH or q in G_tiles:
                    return
                xts = x_pool.tile([128, N_KT, GJT * JW], bf16, tag="xts")
                nc.scalar.dma_start(
                    out=xts, in_=xt_h[:, :, q * GJT * JW : (q + 1) * GJT * JW]
                )
                gt = g_pool.tile([128, GJT, GC], bf16, tag="G", name="G")
                nc.vector.memset(gt[:, :, 0:1], 1.0)
                for s in range(GJT):
                    ps_gw = psum_pool.tile([128, 512], f32, tag="g", bufs=3)
                    ps_g = ps_gw[:, :N_CLS]
                    for t in range(N_KT):
                        nc.tensor.matmul(
                            ps_g,
                            lhsT=xts[:, t, s * JW : (s + 1) * JW],
                            rhs=w2b_sb[:, t, :],
                            start=(t == 0),
                            stop=(t == N_KT - 1),
                        )
                    nc.vector.tensor_add(gt[:, s, 1:GC], ps_g, b2b_sb)
                G_tiles[q] = gt

            # ---- Phase B: stream adjT strip-groups, accumulate out.T ----
            g_job(0)
            for g in range(N_GRP):
                need = (g * GRP) // GJT
                for la in range(1, g_lookahead + 1):
                    g_job(need + la)
                grp = strip_pool.tile([128, GRP, ROWS_PAD], f8, tag="strip")
                gw = min(GRP, N_JT - g * GRP)  # skip all-zero padded j-tiles
                nc.sync.dma_start(out=grp[:, :gw, :], in_=adjT_h[g][:, :gw, :])
                # col-tiled pairs: even jt streams through array cols 0..63,
                # odd jt through cols 64..127, concurrently (M=41 <= 64).
                # Only the very first matmul per bank uses start=True (bank
                # clear); the odd chain's first relies on overwrite-where-
                # unwritten. Emission interleaves A/B per chunk so the PE
                # dispatches both halves back-to-back.
                for s0 in range(0, GRP, 2):
                    jts = [g * GRP + s0 + d for d in range(2)]
                    jts = [j for j in jts if j < N_JT]
                    for ch, (c0, cw) in enumerate(PSU_CHUNKS):
                        for jt in jts:
                            s = jt - g * GRP
                            gt = G_tiles[jt // GJT]
                            gs = jt % GJT
                            po = 64 * (jt % 2)
                            nc.tensor.matmul(
                                psU[ch][po : po + GC, :cw],
                                lhsT=gt[:, gs, :],
                                rhs=grp[:, s, c0 : c0 + cw],
                                tile_position=(0, po),
                                start=(jt <= 1),
                                stop=(jt >= N_JT - 2),
                                skip_group_check=True,
                            )

            # ---- Finalize: transpose back (bf16, 1-pass), divide, bias ----
            # U rows: 0 = deg (ones col), 1..40 = class sums, with the even-jt
            # half at partitions 0..40 and odd-jt half at 64..104. Both halves
            # are copied (DVE / ACT in parallel) into the pre-zeroed U_bf and
            # summed by the transpose matmul against the stacked identity.
            for ch, (c0, cw) in enumerate(PSU_CHUNKS):
                nc.vector.tensor_copy(U_bfs[ch][0:GC, :cw], psU[ch][0:GC, :cw])
                nc.scalar.copy(
                    U_bfs[ch][64 : 64 + GC, :cw], psU[ch][64 : 64 + GC, :cw]
                )
            tiles_f = []
            for ch, (c0, cw) in enumerate(PSU_CHUNKS):
                for k in range(-(-cw // 128)):
                    i0 = c0 + k * 128
                    if i0 >= ROWS:
                        break
                    p = min(128, ROWS - i0, cw - k * 128)
                    tiles_f.append((ch, k, i0, p))
            NFB = 2  # fin PSUM banks
            per_fb = -(-len(tiles_f) // NFB)
            ps_fins = [
                psum_pool.tile([128, per_fb, GC], f32, tag=f"fin{b}", name="fin", bufs=1)
                for b in range(NFB)
            ]
            for n, (ch, k, i0, p) in enumerate(tiles_f):
                ps_fin = ps_fins[n // per_fb]
                nc.tensor.matmul(
                    ps_fin[:p, n % per_fb, :],
                    lhsT=U_bfs[ch][:, k * 128 : k * 128 + p],
                    rhs=id2_b,
                    start=True,
                    stop=True,
                    skip_group_check=True,
                )
            rcps = []
            for b in range(NFB):
                nt = min(per_fb, len(tiles_f) - b * per_fb)
                d1 = out_pool.tile([128, per_fb], f32, tag="d1", name="d1")
                nc.vector.tensor_scalar_add(
                    d1[:, :nt], ps_fins[b][:, :nt, 0], 1.0
                )
                rcp = out_pool.tile([128, per_fb], f32, tag="rcp", name="rcp")
                nc.vector.reciprocal(rcp[:, :nt], d1[:, :nt])
                rcps.append(rcp)
            for n, (ch, k, i0, p) in enumerate(tiles_f):
                b, m = n // per_fb, n % per_fb
                o_sb = out_pool.tile([128, N_CLS], f32, tag="o", name="o")
                nc.vector.scalar_tensor_tensor(
                    out=o_sb[:p],
                    in0=ps_fins[b][:p, m, 1:GC],
                    scalar=rcps[b][:p, m : m + 1],
                    in1=bcls_sb[:p],
                    op0=mult,
                    op1=add,
                )
                eng = nc.sync if n % 2 == 0 else nc.scalar
                eng.dma_start(out=out_h[i0 : i0 + p, :], in_=o_sb[:p])

    nc.compile()
    return nc


_CACHE = {}


def _get_nc():
    if "nc" not in _CACHE:
        _CACHE["nc"] = build_gnn()
    return _CACHE["nc"]


def make_in_maps(x, adj, W_in, b_in, W_cls, b_cls):
    f8 = ml_dtypes.float8_e4m3
    adj8 = np.asarray(adj, dtype=np.float32).astype(f8)
    xp = np.zeros((IN_CH, JPAD), dtype=np.float32)
    xp[:, :N_NODES] = np.asarray(x, dtype=np.float32).T
    x_Ti_full = np.ascontiguousarray(
        xp.reshape(N_KT, 128, JPAD).transpose(1, 0, 2)
    ).astype(ml_dtypes.bfloat16)
    wpack = np.zeros((128, 128 + 128 + N_CLS + 1 + N_CLS + GC), dtype=np.float32)
    wpack[:, 0:128] = np.eye(128, dtype=np.float32)
    wpack[:, 128:256] = (
        np.asarray(W_in, dtype=np.float32)
        .reshape(N_KT, 128, HID)
        .transpose(1, 0, 2)
        .reshape(128, N_KT * HID)
    )
    wpack[:HID, 256 : 256 + N_CLS] = np.asarray(W_cls, dtype=np.float32)
    wpack[:HID, 296] = np.asarray(b_in, dtype=np.float32)
    wpack[:, 297 : 297 + N_CLS] = np.asarray(b_cls, dtype=np.float32)[None, :]
    wpack[0:GC, 337 : 337 + GC] = np.eye(GC, dtype=np.float32)
    wpack[64 : 64 + GC, 337 : 337 + GC] = np.eye(GC, dtype=np.float32)
    in_maps = []
    for c in range(N_CORES):
        sl = slice(c * ROWS, (c + 1) * ROWS)
        blk = np.zeros((JPAD, ROWS_PAD), dtype=f8)
        blk[:N_NODES, :ROWS] = adj8[sl, :].T
        blk = np.ascontiguousarray(
            blk.reshape(N_GRP, GRP, 128, ROWS_PAD).transpose(0, 2, 1, 3)
        )
        in_maps.append({"adjT": blk, "x_Ti": x_Ti_full, "wpack": wpack})
    return in_maps


def kernel(x, adj, W_in, b_in, W_cls, b_cls):
    x = np.asarray(x, dtype=np.float32)
    adj = np.asarray(adj, dtype=np.float32)
    W_in = np.asarray(W_in, dtype=np.float32)
    b_in = np.asarray(b_in, dtype=np.float32)
    W_cls = np.asarray(W_cls, dtype=np.float32)
    b_cls = np.asarray(b_cls, dtype=np.float32)

    nc = _get_nc()
    in_maps = make_in_maps(x, adj, W_in, b_in, W_cls, b_cls)
    res = run_bass_kernel_spmd(nc, in_maps, core_ids=list(range(N_CORES)))
    outs = [res.results[c]["out_blk"] for c in range(N_CORES)]
    return np.concatenate(outs, axis=0).astype(np.float32)
